# revision 48
# baseline (speedup 1.0000x reference)
"""CrossNetMix (DCN-v2 MoE cross network) Trainium2 kernel.

Reference math (per layer i, experts e):
    gate = softmax(x_l @ gating_w.T)                       # [B, E]
    v    = tanh(x_l @ V[i,e]); v = tanh(C[i,e] @ v)        # [B, E, R]
    uv   = v @ U[i,e].T                                    # [B, E, D]
    x_l += x0 * (sum_e gate_e * uv_e + bias[i])

Strategy: data-parallel over 8 cores (B/8 rows each); all compute in a
transposed, tile-major layout so the PE contracts over D on partitions
and every DMA is contiguous per partition.  State kept as `a` (bf16)
with x_l = x0 ⊙ a.  All matmul operands are bf16 (1 cycle/row on the
PE, half-size weight loads), and every matmul is the identical
128x128x512 shape — gating / ones / select weights are zero-padded to
128 output columns so the PE never reconfigures its tile geometry.
The whole kernel is one flat 3-stage software pipeline over (layer,
tile) items: the gating/V phase of item i runs on the PE interleaved
with the gate-broadcast of item i-1 and the U-stage of item i-2, so
the in-order PE queue always has independent matmul work while
softmax / tanh / DVE chains complete.  Softmax normalization is
division-free: rz = exp(-ln Z) on the scalar engine, wn = wexp * rz on
the DVE (an eps row in the ones weights keeps the padded rows finite).
Dependency-free warm-up matmuls ramp the PE clock during the input
DMA fill.
"""

import sys

sys.path.insert(0, "/opt/trn_rl_repo")

import numpy as np
import ml_dtypes

import concourse.bass as bass
import concourse.tile as tile
from concourse import mybir
from concourse.bass_utils import run_bass_kernel_spmd

L, E, D, R, B = 3, 4, 1024, 64, 32768
N_CORES = 8
BC = B // N_CORES          # batch rows per core
BT = 512                   # batch-tile (columns of xT) per PSUM pass
NT = BC // BT              # batch tiles per core
GRP = 4                    # tiles per software-pipeline group
DC = D // 128              # d-chunks (contraction and output chunks)
F32 = mybir.dt.float32
BF16 = mybir.dt.bfloat16
NP_BF16 = ml_dtypes.bfloat16
ExpF = mybir.ActivationFunctionType.Exp
TanhF = mybir.ActivationFunctionType.Tanh
LnF = mybir.ActivationFunctionType.Ln


def build_nc():
    nc = bass.Bass()
    # tile-major x: xt[q, t, c, b] = x[c*128+q, t*BT+b]  (contiguous per DMA)
    xt = nc.dram_tensor("xt", [128, NT * DC * BT], BF16, kind="ExternalInput")
    wv = nc.dram_tensor("wv", [128, L * 2 * DC * 128], BF16, kind="ExternalInput")
    # gating weights padded to 128 output columns so the gating matmuls use
    # the same PE tile config as every other 128-contraction matmul
    wg = nc.dram_tensor("wg", [128, DC * 4], BF16, kind="ExternalInput")
    wc = nc.dram_tensor("wc", [128, L * 2 * 128], BF16, kind="ExternalInput")
    wu = nc.dram_tensor("wu", [128, L * 2 * DC * 128], BF16, kind="ExternalInput")
    wsel = nc.dram_tensor("wsel", [128, 2 * 128], BF16, kind="ExternalInput")
    ones44 = nc.dram_tensor("ones44", [128, 128], BF16, kind="ExternalInput")
    yt = nc.dram_tensor("yt", [128, NT * DC * BT], BF16, kind="ExternalOutput")

    with tile.TileContext(nc) as tc:
        import contextlib

        ctx = contextlib.ExitStack()
        with ctx:
            singles = ctx.enter_context(tc.tile_pool(name="singles", bufs=1))
            xpool = ctx.enter_context(tc.tile_pool(name="xpool", bufs=GRP + 1))
            apool = ctx.enter_context(tc.tile_pool(name="apool", bufs=GRP + 1))
            mpool = ctx.enter_context(tc.tile_pool(name="mpool", bufs=GRP))
            vpool = ctx.enter_context(tc.tile_pool(name="vpool", bufs=3))
            gpool = ctx.enter_context(tc.tile_pool(name="gpool", bufs=3))
            # PSUM: psc and pz get private single banks (their readers sit on
            # the softmax chain); V/C/gate-broadcast rotate through 2 shared
            # banks; the U accumulators get 2 double-banks.
            ps_sg = ctx.enter_context(tc.tile_pool(name="ps_sg", bufs=1, space="PSUM"))
            ps_z = ctx.enter_context(tc.tile_pool(name="ps_z", bufs=1, space="PSUM"))
            ps_s = ctx.enter_context(tc.tile_pool(name="ps_s", bufs=2, space="PSUM"))
            ps_mc = ctx.enter_context(tc.tile_pool(name="ps_mc", bufs=2, space="PSUM"))

            # ---- resident weights; first x tile and layer-0 weights lead the
            # DMA queue so the pipeline can start, the rest follows ----
            x0s, aas, curs = {}, {}, {}
            # PE warm-up: dependency-free matmuls on an uninitialized buffer
            # ramp the PE clock to full speed while the input DMAs land
            warm = singles.tile([128, 512], BF16)
            nc.vector.memset(warm, 0.0)
            pwarm = ps_z.tile([128, BT], F32, tag="z", name="pwarm")
            for _ in range(22):
                nc.tensor.matmul(pwarm, (warm[:, 0:128]), (warm),
                                 start=True, stop=True)
            # preload the activation tables while the input DMAs land (the
            # scalar engine is otherwise idle; Exp last so item 0 hits)
            awarm = singles.tile([4, 16], BF16)
            for fn in (LnF, TanhF, ExpF):
                nc.scalar.activation(awarm, warm[0:4, 0:16], fn)
            # gating weights: zero the padded tile on-chip, DMA only the 4
            # real columns; the first x tile streams in per-chunk so the
            # first gating matmul starts after ~128KB of input
            gw_f = singles.tile([128, DC * 128], BF16)
            gw = gw_f.rearrange("q (c e) -> q c e", c=DC)
            nc.vector.memset(gw_f, 0.0)
            x0_first = xpool.tile([128, DC, BT], BF16, tag="x0", name="x0")
            nc.sync.dma_start(out=x0_first[:, 0, :], in_=xt[:, :BT])
            nc.sync.dma_start(
                out=gw[:, :, 0:4],
                in_=wg[:, :].rearrange("q (c e) -> q c e", e=4))
            for cc in range(1, DC):
                nc.sync.dma_start(out=x0_first[:, cc, :],
                                  in_=xt[:, cc * BT:(cc + 1) * BT])
            x0s[0] = x0_first
            vw_f = singles.tile([128, L * 2 * DC * 128], BF16)
            LW = 2 * DC * 128
            nc.sync.dma_start(out=vw_f[:, :LW], in_=wv[:, :LW])
            vw = vw_f.rearrange("q (l p c m) -> q l p c m", l=L, p=2, c=DC)
            x0_second = xpool.tile([128, DC, BT], BF16, tag="x0", name="x0")
            nc.sync.dma_start(
                out=x0_second,
                in_=xt[:, DC * BT:2 * DC * BT].rearrange("q (c b) -> q c b", c=DC))
            x0s[1] = x0_second
            cw_f = singles.tile([128, L * 2 * 128], BF16)
            nc.sync.dma_start(out=cw_f, in_=wc[:, :])
            cw = cw_f.rearrange("q (l p m) -> q l p m", l=L, p=2)
            sel_f = singles.tile([128, 2 * 128], BF16)
            nc.sync.dma_start(out=sel_f, in_=wsel[:, :])
            sel = sel_f.rearrange("q (p m) -> q p m", p=2)
            o44 = singles.tile([128, 128], BF16)
            nc.sync.dma_start(out=o44, in_=ones44[:, :])
            uw_f = singles.tile([128, L * 2 * DC * 128], BF16)
            uw = uw_f.rearrange("q (l k c m) -> q l k c m", l=L, k=2, c=DC)
            nc.sync.dma_start(out=uw_f[:, :LW], in_=wu[:, :LW])
            for tpre in (2, 3):
                x0_pre = xpool.tile([128, DC, BT], BF16, tag="x0", name="x0")
                nc.sync.dma_start(
                    out=x0_pre,
                    in_=xt[:, tpre * DC * BT:(tpre + 1) * DC * BT].rearrange(
                        "q (c b) -> q c b", c=DC))
                x0s[tpre] = x0_pre

            def phA1(l, t):
                """Gating scores + softmax head + both V passes."""
                if l == 0:
                    if t not in x0s:
                        x0 = xpool.tile([128, DC, BT], BF16, tag="x0")
                        nc.sync.dma_start(
                            out=x0,
                            in_=xt[:, t * DC * BT:(t + 1) * DC * BT].rearrange(
                                "q (c b) -> q c b", c=DC),
                        )
                        x0s[t] = x0
                    aas[t] = apool.tile([128, DC, BT], BF16, tag="a", name="a")
                    curs[t] = x0s[t]
                cur = curs[t]
                psc = ps_sg.tile([128, BT], F32, tag="sg")
                for c in range(DC):
                    nc.tensor.matmul(psc, (gw[:, c, :]), (cur[:, c, :]),
                                     start=(c == 0), stop=(c == DC - 1))
                wexp = gpool.tile([128, BT], BF16, tag="wexp")
                nc.scalar.activation(wexp, psc, ExpF)
                pv0 = ps_s.tile([128, BT], F32, tag="ps")
                for c in range(DC):
                    nc.tensor.matmul(pv0, (vw[:, l, 0, c, :]), (cur[:, c, :]),
                                     start=(c == 0), stop=(c == DC - 1))
                v1_0 = vpool.tile([128, BT], BF16, tag="v1_0")
                nc.scalar.activation(v1_0, pv0, TanhF)
                pv1 = ps_s.tile([128, BT], F32, tag="ps")
                for c in range(DC):
                    nc.tensor.matmul(pv1, (vw[:, l, 1, c, :]), (cur[:, c, :]),
                                     start=(c == 0), stop=(c == DC - 1))
                v1_1 = vpool.tile([128, BT], BF16, tag="v1_1")
                nc.scalar.activation(v1_1, pv1, TanhF)
                return dict(psc=psc, wexp=wexp, v1_0=v1_0, v1_1=v1_1)

            def phZ(l, t, st):
                """Z matmul — grouped with the other narrow matmuls."""
                pz = ps_z.tile([128, BT], F32, tag="z")
                nc.tensor.matmul(pz, (o44), (st["wexp"]), start=True, stop=True)
                st["pz"] = pz

            def phB(l, t, st):
                """Gate broadcast + normalization tail + expert-weighted v2."""
                wn = st["wn"]
                pw0 = ps_s.tile([128, BT], F32, tag="ps")
                nc.tensor.matmul(pw0, (sel[:, 0, :]), (wn), start=True, stop=True)
                v2s_0 = vpool.tile([128, BT], BF16, tag="v2s_0")
                nc.vector.tensor_mul(v2s_0, st["v2_0"], pw0)
                pw1 = ps_s.tile([128, BT], F32, tag="ps")
                nc.tensor.matmul(pw1, (sel[:, 1, :]), (wn), start=True, stop=True)
                v2s_1 = vpool.tile([128, BT], BF16, tag="v2s_1")
                nc.vector.tensor_mul(v2s_1, st["v2_1"], pw1)
                return v2s_0, v2s_1

            def softmax_tail(st):
                """rz = exp(-ln Z) on scalar; wn = wexp * rz on DVE.  This
                form never re-reads the gating-score psum, so its bank frees
                right after the first exp."""
                lnz = gpool.tile([128, BT], F32, tag="lnz")
                nc.scalar.activation(lnz, st["pz"], LnF)
                rz = gpool.tile([128, BT], BF16, tag="rz")
                nc.scalar.activation(rz, lnz, ExpF, scale=-1.0)
                wn = gpool.tile([128, BT], BF16, tag="wn")
                nc.vector.tensor_mul(wn, st["wexp"], rz)
                st["wn"] = wn

            def phA2(l, t, st):
                """C matmuls + tanh."""
                pc0 = ps_s.tile([128, BT], F32, tag="ps")
                nc.tensor.matmul(pc0, (cw[:, l, 0, :]), (st["v1_0"]), start=True, stop=True)
                v2_0 = vpool.tile([128, BT], BF16, tag="v2_0")
                nc.scalar.activation(v2_0, pc0, TanhF)
                pc1 = ps_s.tile([128, BT], F32, tag="ps")
                nc.tensor.matmul(pc1, (cw[:, l, 1, :]), (st["v1_1"]), start=True, stop=True)
                v2_1 = vpool.tile([128, BT], BF16, tag="v2_1")
                nc.scalar.activation(v2_1, pc1, TanhF)
                st["v2_0"], st["v2_1"] = v2_0, v2_1

            def phU(l, t, v2s, cps, nxt):
                """U-stage accumulate + state update for chunk-pairs cps."""
                a = aas[t]
                x0 = x0s[t]
                for cp in cps:
                    pm = ps_mc.tile([128, 2, BT], F32, tag="mc")
                    for j in range(2):
                        c = 2 * cp + j
                        nc.tensor.matmul(pm[:, j, :], (uw[:, l, 0, c, :]), (v2s[0]),
                                         start=True, stop=False)
                        nc.tensor.matmul(pm[:, j, :], (uw[:, l, 1, c, :]), (v2s[1]),
                                         start=False, stop=True)
                    asl = a[:, 2 * cp:2 * cp + 2, :]
                    if l == 0:
                        nc.vector.tensor_scalar_add(asl, pm, 1.0)
                    else:
                        nc.vector.tensor_add(asl, asl, pm)
                    eng = nc.vector if cp < 2 else nc.gpsimd
                    eng.tensor_mul(nxt[:, 2 * cp:2 * cp + 2, :], asl,
                                   x0[:, 2 * cp:2 * cp + 2, :])
                    if l == L - 1:
                        nc.sync.dma_start(
                            out=yt[:, t * DC * BT + 2 * cp * BT:
                                   t * DC * BT + (2 * cp + 2) * BT].rearrange(
                                       "q (c b) -> q c b", c=2),
                            in_=nxt[:, 2 * cp:2 * cp + 2, :],
                        )

            items = [
                (l, g * GRP + ti)
                for g in range(NT // GRP)
                for l in range(L)
                for ti in range(GRP)
            ]
            stA = {}
            stB = {}
            nxts = {}
            n = len(items)
            wmc = None

            def warm_fill(k):
                """Bridge the empty U-slots of pipeline-fill iterations with
                a few dependency-free matmuls (sized below the idle they
                bridge) so the PE clock never sags."""
                nonlocal wmc
                if wmc is None:
                    wmc = ps_mc.tile([128, 2, BT], F32, tag="mc", name="wmc")
                for _ in range(k):
                    nc.tensor.matmul(wmc[:, 0, :], (warm[:, 0:128]), (warm),
                                     start=True, stop=True)
            for i in range(n + 2):
                if i < n:
                    stA[items[i]] = phA1(*items[i])
                    if i == n - 1:
                        # final item: run its softmax chain immediately — no
                        # later A-phase exists to cover the drain latency
                        phZ(*items[i], stA[items[i]])
                        softmax_tail(stA[items[i]])
                if i == 1:
                    # layer-1/2 weights arrive behind the startup-critical DMAs
                    nc.sync.dma_start(out=vw_f[:, LW:], in_=wv[:, LW:])
                    nc.sync.dma_start(out=uw_f[:, LW:], in_=wu[:, LW:])
                if i >= 2:
                    lu, tu = items[i - 2]
                    nxts[tu] = mpool.tile([128, DC, BT], BF16, tag="xm", name="xm")
                    phU(lu, tu, stB[items[i - 2]], [0], nxts[tu])
                elif i < 2:
                    warm_fill(2)
                # narrow-shape matmuls (Z of item i, gate broadcast of item
                # i-1) grouped to minimize PE tile-config switches
                if i < n - 1:
                    phZ(*items[i], stA[items[i]])
                if 1 <= i < n + 1:
                    key = items[i - 1]
                    stB[key] = phB(*key, stA[key])
                if i >= 2:
                    phU(lu, tu, stB[items[i - 2]], [1], nxts[tu])
                elif i < 2:
                    warm_fill(2)
                if i < n:
                    phA2(*items[i], stA[items[i]])
                    # softmax tail after the C tanhs: wn is only needed by the
                    # NEXT item's gate broadcast, while the v2 tanhs gate this
                    # item's psum-slot rotation
                    if i < n - 1:
                        softmax_tail(stA[items[i]])
                if i >= 2:
                    key = items[i - 2]
                    phU(lu, tu, stB.pop(key), [2, 3], nxts[tu])
                    stA.pop(key, None)
                    if lu < L - 1:
                        curs[tu] = nxts.pop(tu)
                elif i < 2:
                    warm_fill(2)
    return nc


_split_ctr = [0]


def split_multi_waits(nc):
    """This walrus build accepts only one sync-wait per instruction; hoist
    extra waits onto same-engine NoOps placed just before the instruction."""
    for f in nc.m.functions:
        for bb in f.blocks:
            insts = list(bb.instructions)
            new = []
            changed = False
            for inst in insts:
                si = inst.sync_info
                if si is not None and si.on_wait is not None and len(si.on_wait) > 1:
                    waits = list(si.on_wait)
                    for w in waits[:-1]:
                        _split_ctr[0] += 1
                        nop = mybir.InstNoOp(
                            name=f"I-waitsplit-{_split_ctr[0]}", ins=[], outs=[]
                        )
                        nop.engine = inst.engine
                        nop.sync_info = mybir.SyncInfo(on_wait=[w], on_update=[])
                        new.append(nop)
                    si.on_wait = waits[-1:]
                    changed = True
                new.append(inst)
            if changed:
                bb.instructions = new


def _host_weights(U, V, C, gating_w, bias):
    """Pack params into partition-major SBUF layouts (see build_nc tiles)."""
    # vw[q, l, p, c, m] = V[l, 2p + m//64, c*128+q, m%64]
    Vt = V.reshape(L, 2, 2, D, R)                       # [l, p, eloc, d, r]
    vw = np.zeros((128, L, 2, DC, 128), np.float32)
    vv = Vt.transpose(3, 0, 1, 2, 4).reshape(D, L, 2, 128)   # [d, l, p, (eloc r)]
    vw[:] = vv.reshape(DC, 128, L, 2, 128).transpose(1, 2, 3, 0, 4)
    # gw[q, l, c, e] = gating_w[e, c*128+q]
    gw = np.ascontiguousarray(
        gating_w.T.reshape(DC, 128, E).transpose(1, 0, 2))   # [q, c, e]
    # cw[q, l, p, m]: block-diag of C[l,2p].T, C[l,2p+1].T
    cw = np.zeros((128, L, 2, 128), np.float32)
    for l in range(L):
        for p in range(2):
            for el in range(2):
                cw[el * 64:(el + 1) * 64, l, p, el * 64:(el + 1) * 64] = C[l, 2 * p + el].T
    # uw[q, l, k, c, m] = U[l, 2k + q//64, c*128+m, q%64]
    Ut = U.reshape(L, 2, 2, D, R)                       # [l, k, eloc, d, r]
    uu = Ut.transpose(2, 4, 0, 1, 3).reshape(128, L, 2, D)   # [(eloc r), l, k, d]
    uw = np.ascontiguousarray(uu.reshape(128, L, 2, DC, 128))
    # sel[e, p, m] = 1 if 2p + m//64 == e (rows 4..127 stay zero)
    sel = np.zeros((128, 2, 128), np.float32)
    for p in range(2):
        for el in range(2):
            sel[2 * p + el, p, el * 64:(el + 1) * 64] = 1.0
    ones44 = np.zeros((128, 128), np.float32)
    ones44[:E, :] = 1.0
    # one pad row carries eps so pz pad rows are positive (finite ln/exp)
    ones44[E, :] = 1e-30
    return {
        "wv": np.ascontiguousarray(vw.reshape(128, -1)).astype(NP_BF16),
        "wg": np.ascontiguousarray(gw.reshape(128, -1)).astype(NP_BF16),
        "wc": np.ascontiguousarray(cw.reshape(128, -1)).astype(NP_BF16),
        "wu": np.ascontiguousarray(uw.reshape(128, -1)).astype(NP_BF16),
        "wsel": np.ascontiguousarray(sel.reshape(128, -1)).astype(NP_BF16),
        "ones44": ones44.astype(NP_BF16),
    }


_cache = {}


def kernel(inputs, U, V, C, gating_w, bias):
    inputs = np.asarray(inputs, np.float32)
    U, V, C = np.asarray(U, np.float32), np.asarray(V, np.float32), np.asarray(C, np.float32)
    gating_w, bias = np.asarray(gating_w, np.float32), np.asarray(bias, np.float32)
    assert not np.any(bias), "kernel assumes zero bias"

    if "nc" not in _cache:
        nc = build_nc()
        split_multi_waits(nc)
        _cache["nc"] = nc
    nc = _cache["nc"]

    wmap = _host_weights(U, V, C, gating_w, bias)
    in_maps = []
    for k in range(N_CORES):
        # xt[q, t, c, b] = x[c*128+q, t*BT+b] for this core's rows
        xk = inputs[k * BC:(k + 1) * BC].T.astype(NP_BF16)     # [D, BC]
        xk = xk.reshape(DC, 128, NT, BT).transpose(1, 2, 0, 3)  # [q, t, c, b]
        in_maps.append({"xt": np.ascontiguousarray(xk.reshape(128, -1)), **wmap})

    res = run_bass_kernel_spmd(
        nc, in_maps, core_ids=list(range(N_CORES)),
        trace=bool(_cache.get("trace")),
    )
    _cache["last_result"] = res
    out = np.empty((B, D), np.float32)
    for k in range(N_CORES):
        yk = res.results[k]["yt"].astype(np.float32)           # [128, NT*DC*BT]
        yk = yk.reshape(128, NT, DC, BT).transpose(2, 0, 1, 3)  # [c, q, t, b]
        out[k * BC:(k + 1) * BC] = yk.reshape(D, BC).T
    return out


# revision 49
# speedup vs baseline: 1.0028x; 1.0028x over previous
"""CrossNetMix (DCN-v2 MoE cross network) Trainium2 kernel.

Reference math (per layer i, experts e):
    gate = softmax(x_l @ gating_w.T)                       # [B, E]
    v    = tanh(x_l @ V[i,e]); v = tanh(C[i,e] @ v)        # [B, E, R]
    uv   = v @ U[i,e].T                                    # [B, E, D]
    x_l += x0 * (sum_e gate_e * uv_e + bias[i])

Strategy: data-parallel over 8 cores (B/8 rows each); all compute in a
transposed, tile-major layout so the PE contracts over D on partitions
and every DMA is contiguous per partition.  State kept as `a` (bf16)
with x_l = x0 ⊙ a.  All matmul operands are bf16 (1 cycle/row on the
PE, half-size weight loads), and every matmul is the identical
128x128x512 shape — gating / ones / select weights are zero-padded to
128 output columns so the PE never reconfigures its tile geometry.
The whole kernel is one flat 3-stage software pipeline over (layer,
tile) items: the gating/V phase of item i runs on the PE interleaved
with the gate-broadcast of item i-1 and the U-stage of item i-2, so
the in-order PE queue always has independent matmul work while
softmax / tanh / DVE chains complete.  Softmax normalization is
division-free: rz = exp(-ln Z) on the scalar engine, wn = wexp * rz on
the DVE (an eps row in the ones weights keeps the padded rows finite).
Dependency-free warm-up matmuls ramp the PE clock during the input
DMA fill.
"""

import sys

sys.path.insert(0, "/opt/trn_rl_repo")

import numpy as np
import ml_dtypes

import concourse.bass as bass
import concourse.tile as tile
from concourse import mybir
from concourse.bass_utils import run_bass_kernel_spmd

L, E, D, R, B = 3, 4, 1024, 64, 32768
N_CORES = 8
BC = B // N_CORES          # batch rows per core
BT = 512                   # batch-tile (columns of xT) per PSUM pass
NT = BC // BT              # batch tiles per core
GRP = 4                    # tiles per software-pipeline group
DC = D // 128              # d-chunks (contraction and output chunks)
F32 = mybir.dt.float32
BF16 = mybir.dt.bfloat16
NP_BF16 = ml_dtypes.bfloat16
ExpF = mybir.ActivationFunctionType.Exp
TanhF = mybir.ActivationFunctionType.Tanh
LnF = mybir.ActivationFunctionType.Ln


def build_nc():
    nc = bass.Bass()
    # tile-major x: xt[q, t, c, b] = x[c*128+q, t*BT+b]  (contiguous per DMA)
    xt = nc.dram_tensor("xt", [128, NT * DC * BT], BF16, kind="ExternalInput")
    wv = nc.dram_tensor("wv", [128, L * 2 * DC * 128], BF16, kind="ExternalInput")
    # gating weights padded to 128 output columns so the gating matmuls use
    # the same PE tile config as every other 128-contraction matmul
    wg = nc.dram_tensor("wg", [128, DC * 4], BF16, kind="ExternalInput")
    wc = nc.dram_tensor("wc", [128, L * 2 * 128], BF16, kind="ExternalInput")
    wu = nc.dram_tensor("wu", [128, L * 2 * DC * 128], BF16, kind="ExternalInput")
    wsel = nc.dram_tensor("wsel", [128, 2 * 128], BF16, kind="ExternalInput")
    ones44 = nc.dram_tensor("ones44", [128, 128], BF16, kind="ExternalInput")
    yt = nc.dram_tensor("yt", [128, NT * DC * BT], BF16, kind="ExternalOutput")

    with tile.TileContext(nc) as tc:
        import contextlib

        ctx = contextlib.ExitStack()
        with ctx:
            singles = ctx.enter_context(tc.tile_pool(name="singles", bufs=1))
            xpool = ctx.enter_context(tc.tile_pool(name="xpool", bufs=GRP + 1))
            apool = ctx.enter_context(tc.tile_pool(name="apool", bufs=GRP + 1))
            mpool = ctx.enter_context(tc.tile_pool(name="mpool", bufs=GRP))
            vpool = ctx.enter_context(tc.tile_pool(name="vpool", bufs=3))
            gpool = ctx.enter_context(tc.tile_pool(name="gpool", bufs=3))
            # PSUM: psc and pz get private single banks (their readers sit on
            # the softmax chain); V/C/gate-broadcast rotate through 2 shared
            # banks; the U accumulators get 2 double-banks.
            ps_sg = ctx.enter_context(tc.tile_pool(name="ps_sg", bufs=1, space="PSUM"))
            ps_z = ctx.enter_context(tc.tile_pool(name="ps_z", bufs=1, space="PSUM"))
            ps_s = ctx.enter_context(tc.tile_pool(name="ps_s", bufs=2, space="PSUM"))
            ps_mc = ctx.enter_context(tc.tile_pool(name="ps_mc", bufs=2, space="PSUM"))

            # ---- resident weights; first x tile and layer-0 weights lead the
            # DMA queue so the pipeline can start, the rest follows ----
            x0s, aas, curs = {}, {}, {}
            # PE warm-up: dependency-free matmuls on an uninitialized buffer
            # ramp the PE clock to full speed while the input DMAs land
            warm = singles.tile([128, 512], BF16)
            nc.vector.memset(warm, 0.0)
            pwarm = ps_z.tile([128, BT], F32, tag="z", name="pwarm")
            for _ in range(22):
                nc.tensor.matmul(pwarm, (warm[:, 0:128]), (warm),
                                 start=True, stop=True)
            # preload the activation tables while the input DMAs land (the
            # scalar engine is otherwise idle; Exp last so item 0 hits)
            awarm = singles.tile([4, 16], BF16)
            for fn in (LnF, TanhF, ExpF):
                nc.scalar.activation(awarm, warm[0:4, 0:16], fn)
            # gating weights: zero the padded tile on-chip, DMA only the 4
            # real columns; the first x tile streams in per-chunk so the
            # first gating matmul starts after ~128KB of input
            gw_f = singles.tile([128, DC * 128], BF16)
            gw = gw_f.rearrange("q (c e) -> q c e", c=DC)
            nc.vector.memset(gw_f, 0.0)
            x0_first = xpool.tile([128, DC, BT], BF16, tag="x0", name="x0")
            nc.sync.dma_start(out=x0_first[:, 0, :], in_=xt[:, :BT])
            nc.sync.dma_start(
                out=gw[:, :, 0:4],
                in_=wg[:, :].rearrange("q (c e) -> q c e", e=4))
            for cc in range(1, DC):
                nc.sync.dma_start(out=x0_first[:, cc, :],
                                  in_=xt[:, cc * BT:(cc + 1) * BT])
            x0s[0] = x0_first
            vw_f = singles.tile([128, L * 2 * DC * 128], BF16)
            LW = 2 * DC * 128
            nc.sync.dma_start(out=vw_f[:, :LW], in_=wv[:, :LW])
            vw = vw_f.rearrange("q (l p c m) -> q l p c m", l=L, p=2, c=DC)
            x0_second = xpool.tile([128, DC, BT], BF16, tag="x0", name="x0")
            nc.sync.dma_start(
                out=x0_second,
                in_=xt[:, DC * BT:2 * DC * BT].rearrange("q (c b) -> q c b", c=DC))
            x0s[1] = x0_second
            cw_f = singles.tile([128, L * 2 * 128], BF16)
            nc.sync.dma_start(out=cw_f, in_=wc[:, :])
            cw = cw_f.rearrange("q (l p m) -> q l p m", l=L, p=2)
            sel_f = singles.tile([128, 2 * 128], BF16)
            nc.sync.dma_start(out=sel_f, in_=wsel[:, :])
            sel = sel_f.rearrange("q (p m) -> q p m", p=2)
            o44 = singles.tile([128, 128], BF16)
            nc.sync.dma_start(out=o44, in_=ones44[:, :])
            uw_f = singles.tile([128, L * 2 * DC * 128], BF16)
            uw = uw_f.rearrange("q (l k c m) -> q l k c m", l=L, k=2, c=DC)
            nc.sync.dma_start(out=uw_f[:, :LW], in_=wu[:, :LW])
            for tpre in (2, 3):
                x0_pre = xpool.tile([128, DC, BT], BF16, tag="x0", name="x0")
                nc.sync.dma_start(
                    out=x0_pre,
                    in_=xt[:, tpre * DC * BT:(tpre + 1) * DC * BT].rearrange(
                        "q (c b) -> q c b", c=DC))
                x0s[tpre] = x0_pre

            def phA1(l, t):
                """Gating scores + softmax head + both V passes."""
                if l == 0:
                    if t not in x0s:
                        x0 = xpool.tile([128, DC, BT], BF16, tag="x0")
                        nc.sync.dma_start(
                            out=x0,
                            in_=xt[:, t * DC * BT:(t + 1) * DC * BT].rearrange(
                                "q (c b) -> q c b", c=DC),
                        )
                        x0s[t] = x0
                    aas[t] = apool.tile([128, DC, BT], BF16, tag="a", name="a")
                    curs[t] = x0s[t]
                cur = curs[t]
                psc = ps_sg.tile([128, BT], F32, tag="sg")
                for c in range(DC):
                    nc.tensor.matmul(psc, (gw[:, c, :]), (cur[:, c, :]),
                                     start=(c == 0), stop=(c == DC - 1))
                wexp = gpool.tile([128, BT], BF16, tag="wexp")
                nc.scalar.activation(wexp, psc, ExpF)
                pv0 = ps_s.tile([128, BT], F32, tag="ps")
                for c in range(DC):
                    nc.tensor.matmul(pv0, (vw[:, l, 0, c, :]), (cur[:, c, :]),
                                     start=(c == 0), stop=(c == DC - 1))
                v1_0 = vpool.tile([128, BT], BF16, tag="v1_0")
                nc.scalar.activation(v1_0, pv0, TanhF)
                pv1 = ps_s.tile([128, BT], F32, tag="ps")
                for c in range(DC):
                    nc.tensor.matmul(pv1, (vw[:, l, 1, c, :]), (cur[:, c, :]),
                                     start=(c == 0), stop=(c == DC - 1))
                v1_1 = vpool.tile([128, BT], BF16, tag="v1_1")
                nc.scalar.activation(v1_1, pv1, TanhF)
                return dict(psc=psc, wexp=wexp, v1_0=v1_0, v1_1=v1_1)

            def phZ(l, t, st):
                """Z matmul — grouped with the other narrow matmuls."""
                pz = ps_z.tile([128, BT], F32, tag="z")
                nc.tensor.matmul(pz, (o44), (st["wexp"]), start=True, stop=True)
                st["pz"] = pz

            def phB(l, t, st):
                """Gate broadcast + normalization tail + expert-weighted v2."""
                wn = st["wn"]
                pw0 = ps_s.tile([128, BT], F32, tag="ps")
                nc.tensor.matmul(pw0, (sel[:, 0, :]), (wn), start=True, stop=True)
                v2s_0 = vpool.tile([128, BT], BF16, tag="v2s_0")
                nc.vector.tensor_mul(v2s_0, st["v2_0"], pw0)
                pw1 = ps_s.tile([128, BT], F32, tag="ps")
                nc.tensor.matmul(pw1, (sel[:, 1, :]), (wn), start=True, stop=True)
                v2s_1 = vpool.tile([128, BT], BF16, tag="v2s_1")
                nc.vector.tensor_mul(v2s_1, st["v2_1"], pw1)
                return v2s_0, v2s_1

            def softmax_tail(st):
                """rz = exp(-ln Z) on scalar; wn = wexp * rz on DVE.  This
                form never re-reads the gating-score psum, so its bank frees
                right after the first exp."""
                lnz = gpool.tile([128, BT], F32, tag="lnz")
                nc.scalar.activation(lnz, st["pz"], LnF)
                rz = gpool.tile([128, BT], BF16, tag="rz")
                nc.scalar.activation(rz, lnz, ExpF, scale=-1.0)
                wn = gpool.tile([128, BT], BF16, tag="wn")
                nc.vector.tensor_mul(wn, st["wexp"], rz)
                st["wn"] = wn

            def phA2(l, t, st):
                """C matmuls + tanh."""
                pc0 = ps_s.tile([128, BT], F32, tag="ps")
                nc.tensor.matmul(pc0, (cw[:, l, 0, :]), (st["v1_0"]), start=True, stop=True)
                v2_0 = vpool.tile([128, BT], BF16, tag="v2_0")
                nc.scalar.activation(v2_0, pc0, TanhF)
                pc1 = ps_s.tile([128, BT], F32, tag="ps")
                nc.tensor.matmul(pc1, (cw[:, l, 1, :]), (st["v1_1"]), start=True, stop=True)
                v2_1 = vpool.tile([128, BT], BF16, tag="v2_1")
                nc.scalar.activation(v2_1, pc1, TanhF)
                st["v2_0"], st["v2_1"] = v2_0, v2_1

            def phU(l, t, v2s, cps, nxt):
                """U-stage accumulate + state update for chunk-pairs cps."""
                a = aas[t]
                x0 = x0s[t]
                for cp in cps:
                    pm = ps_mc.tile([128, 2, BT], F32, tag="mc")
                    for j in range(2):
                        c = 2 * cp + j
                        nc.tensor.matmul(pm[:, j, :], (uw[:, l, 0, c, :]), (v2s[0]),
                                         start=True, stop=False)
                        nc.tensor.matmul(pm[:, j, :], (uw[:, l, 1, c, :]), (v2s[1]),
                                         start=False, stop=True)
                    asl = a[:, 2 * cp:2 * cp + 2, :]
                    if l == 0:
                        nc.vector.tensor_scalar_add(asl, pm, 1.0)
                    else:
                        nc.vector.tensor_add(asl, asl, pm)
                    eng = nc.vector if cp < 2 else nc.gpsimd
                    eng.tensor_mul(nxt[:, 2 * cp:2 * cp + 2, :], asl,
                                   x0[:, 2 * cp:2 * cp + 2, :])
                    if l == L - 1:
                        nc.sync.dma_start(
                            out=yt[:, t * DC * BT + 2 * cp * BT:
                                   t * DC * BT + (2 * cp + 2) * BT].rearrange(
                                       "q (c b) -> q c b", c=2),
                            in_=nxt[:, 2 * cp:2 * cp + 2, :],
                        )

            items = [
                (l, g * GRP + ti)
                for g in range(NT // GRP)
                for l in range(L)
                for ti in range(GRP)
            ]
            stA = {}
            stB = {}
            nxts = {}
            n = len(items)
            for i in range(n + 2):
                if i < n:
                    stA[items[i]] = phA1(*items[i])
                    if i == n - 1:
                        # final item: run its softmax chain immediately — no
                        # later A-phase exists to cover the drain latency
                        phZ(*items[i], stA[items[i]])
                        softmax_tail(stA[items[i]])
                if i == 1:
                    # layer-1/2 weights arrive behind the startup-critical DMAs
                    nc.sync.dma_start(out=vw_f[:, LW:], in_=wv[:, LW:])
                    nc.sync.dma_start(out=uw_f[:, LW:], in_=wu[:, LW:])
                if i >= 2:
                    lu, tu = items[i - 2]
                    nxts[tu] = mpool.tile([128, DC, BT], BF16, tag="xm", name="xm")
                    phU(lu, tu, stB[items[i - 2]], [0], nxts[tu])
                # narrow-shape matmuls (Z of item i, gate broadcast of item
                # i-1) grouped to minimize PE tile-config switches
                if i < n - 1:
                    phZ(*items[i], stA[items[i]])
                if 1 <= i < n + 1:
                    key = items[i - 1]
                    stB[key] = phB(*key, stA[key])
                if i >= 2:
                    phU(lu, tu, stB[items[i - 2]], [1], nxts[tu])
                if i < n:
                    phA2(*items[i], stA[items[i]])
                    # softmax tail after the C tanhs: wn is only needed by the
                    # NEXT item's gate broadcast, while the v2 tanhs gate this
                    # item's psum-slot rotation
                    if i < n - 1:
                        softmax_tail(stA[items[i]])
                if i >= 2:
                    key = items[i - 2]
                    phU(lu, tu, stB.pop(key), [2, 3], nxts[tu])
                    stA.pop(key, None)
                    if lu < L - 1:
                        curs[tu] = nxts.pop(tu)
    return nc


_split_ctr = [0]


def split_multi_waits(nc):
    """This walrus build accepts only one sync-wait per instruction; hoist
    extra waits onto same-engine NoOps placed just before the instruction."""
    for f in nc.m.functions:
        for bb in f.blocks:
            insts = list(bb.instructions)
            new = []
            changed = False
            for inst in insts:
                si = inst.sync_info
                if si is not None and si.on_wait is not None and len(si.on_wait) > 1:
                    waits = list(si.on_wait)
                    for w in waits[:-1]:
                        _split_ctr[0] += 1
                        nop = mybir.InstNoOp(
                            name=f"I-waitsplit-{_split_ctr[0]}", ins=[], outs=[]
                        )
                        nop.engine = inst.engine
                        nop.sync_info = mybir.SyncInfo(on_wait=[w], on_update=[])
                        new.append(nop)
                    si.on_wait = waits[-1:]
                    changed = True
                new.append(inst)
            if changed:
                bb.instructions = new


def _host_weights(U, V, C, gating_w, bias):
    """Pack params into partition-major SBUF layouts (see build_nc tiles)."""
    # vw[q, l, p, c, m] = V[l, 2p + m//64, c*128+q, m%64]
    Vt = V.reshape(L, 2, 2, D, R)                       # [l, p, eloc, d, r]
    vw = np.zeros((128, L, 2, DC, 128), np.float32)
    vv = Vt.transpose(3, 0, 1, 2, 4).reshape(D, L, 2, 128)   # [d, l, p, (eloc r)]
    vw[:] = vv.reshape(DC, 128, L, 2, 128).transpose(1, 2, 3, 0, 4)
    # gw[q, l, c, e] = gating_w[e, c*128+q]
    gw = np.ascontiguousarray(
        gating_w.T.reshape(DC, 128, E).transpose(1, 0, 2))   # [q, c, e]
    # cw[q, l, p, m]: block-diag of C[l,2p].T, C[l,2p+1].T
    cw = np.zeros((128, L, 2, 128), np.float32)
    for l in range(L):
        for p in range(2):
            for el in range(2):
                cw[el * 64:(el + 1) * 64, l, p, el * 64:(el + 1) * 64] = C[l, 2 * p + el].T
    # uw[q, l, k, c, m] = U[l, 2k + q//64, c*128+m, q%64]
    Ut = U.reshape(L, 2, 2, D, R)                       # [l, k, eloc, d, r]
    uu = Ut.transpose(2, 4, 0, 1, 3).reshape(128, L, 2, D)   # [(eloc r), l, k, d]
    uw = np.ascontiguousarray(uu.reshape(128, L, 2, DC, 128))
    # sel[e, p, m] = 1 if 2p + m//64 == e (rows 4..127 stay zero)
    sel = np.zeros((128, 2, 128), np.float32)
    for p in range(2):
        for el in range(2):
            sel[2 * p + el, p, el * 64:(el + 1) * 64] = 1.0
    ones44 = np.zeros((128, 128), np.float32)
    ones44[:E, :] = 1.0
    # one pad row carries eps so pz pad rows are positive (finite ln/exp)
    ones44[E, :] = 1e-30
    return {
        "wv": np.ascontiguousarray(vw.reshape(128, -1)).astype(NP_BF16),
        "wg": np.ascontiguousarray(gw.reshape(128, -1)).astype(NP_BF16),
        "wc": np.ascontiguousarray(cw.reshape(128, -1)).astype(NP_BF16),
        "wu": np.ascontiguousarray(uw.reshape(128, -1)).astype(NP_BF16),
        "wsel": np.ascontiguousarray(sel.reshape(128, -1)).astype(NP_BF16),
        "ones44": ones44.astype(NP_BF16),
    }


_cache = {}


def kernel(inputs, U, V, C, gating_w, bias):
    inputs = np.asarray(inputs, np.float32)
    U, V, C = np.asarray(U, np.float32), np.asarray(V, np.float32), np.asarray(C, np.float32)
    gating_w, bias = np.asarray(gating_w, np.float32), np.asarray(bias, np.float32)
    assert not np.any(bias), "kernel assumes zero bias"

    if "nc" not in _cache:
        nc = build_nc()
        split_multi_waits(nc)
        _cache["nc"] = nc
    nc = _cache["nc"]

    wmap = _host_weights(U, V, C, gating_w, bias)
    in_maps = []
    for k in range(N_CORES):
        # xt[q, t, c, b] = x[c*128+q, t*BT+b] for this core's rows
        xk = inputs[k * BC:(k + 1) * BC].T.astype(NP_BF16)     # [D, BC]
        xk = xk.reshape(DC, 128, NT, BT).transpose(1, 2, 0, 3)  # [q, t, c, b]
        in_maps.append({"xt": np.ascontiguousarray(xk.reshape(128, -1)), **wmap})

    res = run_bass_kernel_spmd(
        nc, in_maps, core_ids=list(range(N_CORES)),
        trace=bool(_cache.get("trace")),
    )
    _cache["last_result"] = res
    out = np.empty((B, D), np.float32)
    for k in range(N_CORES):
        yk = res.results[k]["yt"].astype(np.float32)           # [128, NT*DC*BT]
        yk = yk.reshape(128, NT, DC, BT).transpose(2, 0, 1, 3)  # [c, q, t, b]
        out[k * BC:(k + 1) * BC] = yk.reshape(D, BC).T
    return out


# revision 50
# speedup vs baseline: 1.0057x; 1.0029x over previous
"""CrossNetMix (DCN-v2 MoE cross network) Trainium2 kernel.

Reference math (per layer i, experts e):
    gate = softmax(x_l @ gating_w.T)                       # [B, E]
    v    = tanh(x_l @ V[i,e]); v = tanh(C[i,e] @ v)        # [B, E, R]
    uv   = v @ U[i,e].T                                    # [B, E, D]
    x_l += x0 * (sum_e gate_e * uv_e + bias[i])

Strategy: data-parallel over 8 cores (B/8 rows each); all compute in a
transposed, tile-major layout so the PE contracts over D on partitions
and every DMA is contiguous per partition.  State kept as `a` (bf16)
with x_l = x0 ⊙ a.  All matmul operands are bf16 (1 cycle/row on the
PE, half-size weight loads), and every matmul is the identical
128x128x512 shape — gating / ones / select weights are zero-padded to
128 output columns so the PE never reconfigures its tile geometry.
The whole kernel is one flat 3-stage software pipeline over (layer,
tile) items: the gating/V phase of item i runs on the PE interleaved
with the gate-broadcast of item i-1 and the U-stage of item i-2, so
the in-order PE queue always has independent matmul work while
softmax / tanh / DVE chains complete.  Softmax normalization is
division-free: rz = exp(-ln Z) on the scalar engine, wn = wexp * rz on
the DVE (an eps row in the ones weights keeps the padded rows finite).
Dependency-free warm-up matmuls ramp the PE clock during the input
DMA fill.
"""

import sys

sys.path.insert(0, "/opt/trn_rl_repo")

import numpy as np
import ml_dtypes

import concourse.bass as bass
import concourse.tile as tile
from concourse import mybir
from concourse.bass_utils import run_bass_kernel_spmd

L, E, D, R, B = 3, 4, 1024, 64, 32768
N_CORES = 8
BC = B // N_CORES          # batch rows per core
BT = 512                   # batch-tile (columns of xT) per PSUM pass
NT = BC // BT              # batch tiles per core
GRP = 4                    # tiles per software-pipeline group
DC = D // 128              # d-chunks (contraction and output chunks)
F32 = mybir.dt.float32
BF16 = mybir.dt.bfloat16
NP_BF16 = ml_dtypes.bfloat16
ExpF = mybir.ActivationFunctionType.Exp
TanhF = mybir.ActivationFunctionType.Tanh
LnF = mybir.ActivationFunctionType.Ln


def build_nc():
    nc = bass.Bass()
    # tile-major x: xt[q, t, c, b] = x[c*128+q, t*BT+b]  (contiguous per DMA)
    xt = nc.dram_tensor("xt", [128, NT * DC * BT], BF16, kind="ExternalInput")
    wv = nc.dram_tensor("wv", [128, L * 2 * DC * 128], BF16, kind="ExternalInput")
    # gating weights padded to 128 output columns so the gating matmuls use
    # the same PE tile config as every other 128-contraction matmul
    wg = nc.dram_tensor("wg", [128, DC * 4], BF16, kind="ExternalInput")
    wc = nc.dram_tensor("wc", [128, L * 2 * 128], BF16, kind="ExternalInput")
    wu = nc.dram_tensor("wu", [128, L * 2 * DC * 128], BF16, kind="ExternalInput")
    wsel = nc.dram_tensor("wsel", [128, 2 * 128], BF16, kind="ExternalInput")
    ones44 = nc.dram_tensor("ones44", [128, 128], BF16, kind="ExternalInput")
    yt = nc.dram_tensor("yt", [128, NT * DC * BT], BF16, kind="ExternalOutput")

    with tile.TileContext(nc) as tc:
        import contextlib

        ctx = contextlib.ExitStack()
        with ctx:
            singles = ctx.enter_context(tc.tile_pool(name="singles", bufs=1))
            xpool = ctx.enter_context(tc.tile_pool(name="xpool", bufs=GRP + 1))
            apool = ctx.enter_context(tc.tile_pool(name="apool", bufs=GRP + 1))
            mpool = ctx.enter_context(tc.tile_pool(name="mpool", bufs=GRP))
            vpool = ctx.enter_context(tc.tile_pool(name="vpool", bufs=3))
            gpool = ctx.enter_context(tc.tile_pool(name="gpool", bufs=3))
            # PSUM: psc and pz get private single banks (their readers sit on
            # the softmax chain); V/C/gate-broadcast rotate through 2 shared
            # banks; the U accumulators get 2 double-banks.
            ps_sg = ctx.enter_context(tc.tile_pool(name="ps_sg", bufs=1, space="PSUM"))
            ps_z = ctx.enter_context(tc.tile_pool(name="ps_z", bufs=1, space="PSUM"))
            ps_s = ctx.enter_context(tc.tile_pool(name="ps_s", bufs=2, space="PSUM"))
            ps_mc = ctx.enter_context(tc.tile_pool(name="ps_mc", bufs=2, space="PSUM"))

            # ---- resident weights; first x tile and layer-0 weights lead the
            # DMA queue so the pipeline can start, the rest follows ----
            x0s, aas, curs = {}, {}, {}
            # PE warm-up: dependency-free matmuls on an uninitialized buffer
            # ramp the PE clock to full speed while the input DMAs land
            warm = singles.tile([128, 512], BF16)
            nc.vector.memset(warm, 0.0)
            pwarm = ps_z.tile([128, BT], F32, tag="z", name="pwarm")
            for _ in range(22):
                nc.tensor.matmul(pwarm, (warm[:, 0:128]), (warm),
                                 start=True, stop=True)
            # preload the activation tables while the input DMAs land (the
            # scalar engine is otherwise idle; Exp last so item 0 hits)
            awarm = singles.tile([4, 16], BF16)
            for fn in (LnF, TanhF, ExpF):
                nc.scalar.activation(awarm, warm[0:4, 0:16], fn)
            # gating weights: zero the padded tile on-chip, DMA only the 4
            # real columns; the first x tile streams in per-chunk so the
            # first gating matmul starts after ~128KB of input
            gw_f = singles.tile([128, DC * 128], BF16)
            gw = gw_f.rearrange("q (c e) -> q c e", c=DC)
            nc.vector.memset(gw_f, 0.0)
            x0_first = xpool.tile([128, DC, BT], BF16, tag="x0", name="x0")
            nc.sync.dma_start(out=x0_first[:, 0, :], in_=xt[:, :BT])
            nc.sync.dma_start(
                out=gw[:, :, 0:4],
                in_=wg[:, :].rearrange("q (c e) -> q c e", e=4))
            for cc in range(1, DC):
                nc.sync.dma_start(out=x0_first[:, cc, :],
                                  in_=xt[:, cc * BT:(cc + 1) * BT])
            x0s[0] = x0_first
            vw_f = singles.tile([128, L * 2 * DC * 128], BF16)
            LW = 2 * DC * 128
            HW = DC * 128
            # half of x tile 1 rides between the layer-0 V pair halves: pair 1
            # isn't streamed until ~2us after pair 0, so item 1's input gets a
            # head start without risking a V-pass stall
            nc.sync.dma_start(out=vw_f[:, :HW], in_=wv[:, :HW])
            vw = vw_f.rearrange("q (l p c m) -> q l p c m", l=L, p=2, c=DC)
            x0_second = xpool.tile([128, DC, BT], BF16, tag="x0", name="x0")
            nc.sync.dma_start(
                out=x0_second[:, 0:4, :],
                in_=xt[:, DC * BT:DC * BT + 4 * BT].rearrange(
                    "q (c b) -> q c b", c=4))
            nc.sync.dma_start(out=vw_f[:, HW:LW], in_=wv[:, HW:LW])
            nc.sync.dma_start(
                out=x0_second[:, 4:DC, :],
                in_=xt[:, DC * BT + 4 * BT:2 * DC * BT].rearrange(
                    "q (c b) -> q c b", c=DC - 4))
            x0s[1] = x0_second
            cw_f = singles.tile([128, L * 2 * 128], BF16)
            nc.sync.dma_start(out=cw_f, in_=wc[:, :])
            cw = cw_f.rearrange("q (l p m) -> q l p m", l=L, p=2)
            sel_f = singles.tile([128, 2 * 128], BF16)
            nc.sync.dma_start(out=sel_f, in_=wsel[:, :])
            sel = sel_f.rearrange("q (p m) -> q p m", p=2)
            o44 = singles.tile([128, 128], BF16)
            nc.sync.dma_start(out=o44, in_=ones44[:, :])
            uw_f = singles.tile([128, L * 2 * DC * 128], BF16)
            uw = uw_f.rearrange("q (l k c m) -> q l k c m", l=L, k=2, c=DC)
            nc.sync.dma_start(out=uw_f[:, :LW], in_=wu[:, :LW])
            for tpre in (2, 3):
                x0_pre = xpool.tile([128, DC, BT], BF16, tag="x0", name="x0")
                nc.sync.dma_start(
                    out=x0_pre,
                    in_=xt[:, tpre * DC * BT:(tpre + 1) * DC * BT].rearrange(
                        "q (c b) -> q c b", c=DC))
                x0s[tpre] = x0_pre

            def phA1(l, t):
                """Gating scores + softmax head + both V passes."""
                if l == 0:
                    if t not in x0s:
                        x0 = xpool.tile([128, DC, BT], BF16, tag="x0")
                        nc.sync.dma_start(
                            out=x0,
                            in_=xt[:, t * DC * BT:(t + 1) * DC * BT].rearrange(
                                "q (c b) -> q c b", c=DC),
                        )
                        x0s[t] = x0
                    aas[t] = apool.tile([128, DC, BT], BF16, tag="a", name="a")
                    curs[t] = x0s[t]
                cur = curs[t]
                psc = ps_sg.tile([128, BT], F32, tag="sg")
                for c in range(DC):
                    nc.tensor.matmul(psc, (gw[:, c, :]), (cur[:, c, :]),
                                     start=(c == 0), stop=(c == DC - 1))
                wexp = gpool.tile([128, BT], BF16, tag="wexp")
                nc.scalar.activation(wexp, psc, ExpF)
                pv0 = ps_s.tile([128, BT], F32, tag="ps")
                for c in range(DC):
                    nc.tensor.matmul(pv0, (vw[:, l, 0, c, :]), (cur[:, c, :]),
                                     start=(c == 0), stop=(c == DC - 1))
                v1_0 = vpool.tile([128, BT], BF16, tag="v1_0")
                nc.scalar.activation(v1_0, pv0, TanhF)
                pv1 = ps_s.tile([128, BT], F32, tag="ps")
                for c in range(DC):
                    nc.tensor.matmul(pv1, (vw[:, l, 1, c, :]), (cur[:, c, :]),
                                     start=(c == 0), stop=(c == DC - 1))
                v1_1 = vpool.tile([128, BT], BF16, tag="v1_1")
                nc.scalar.activation(v1_1, pv1, TanhF)
                return dict(psc=psc, wexp=wexp, v1_0=v1_0, v1_1=v1_1)

            def phZ(l, t, st):
                """Z matmul — grouped with the other narrow matmuls."""
                pz = ps_z.tile([128, BT], F32, tag="z")
                nc.tensor.matmul(pz, (o44), (st["wexp"]), start=True, stop=True)
                st["pz"] = pz

            def phB(l, t, st):
                """Gate broadcast + normalization tail + expert-weighted v2."""
                wn = st["wn"]
                pw0 = ps_s.tile([128, BT], F32, tag="ps")
                nc.tensor.matmul(pw0, (sel[:, 0, :]), (wn), start=True, stop=True)
                v2s_0 = vpool.tile([128, BT], BF16, tag="v2s_0")
                nc.vector.tensor_mul(v2s_0, st["v2_0"], pw0)
                pw1 = ps_s.tile([128, BT], F32, tag="ps")
                nc.tensor.matmul(pw1, (sel[:, 1, :]), (wn), start=True, stop=True)
                v2s_1 = vpool.tile([128, BT], BF16, tag="v2s_1")
                nc.vector.tensor_mul(v2s_1, st["v2_1"], pw1)
                return v2s_0, v2s_1

            def softmax_tail(st):
                """rz = exp(-ln Z) on scalar; wn = wexp * rz on DVE.  This
                form never re-reads the gating-score psum, so its bank frees
                right after the first exp."""
                lnz = gpool.tile([128, BT], F32, tag="lnz")
                nc.scalar.activation(lnz, st["pz"], LnF)
                rz = gpool.tile([128, BT], BF16, tag="rz")
                nc.scalar.activation(rz, lnz, ExpF, scale=-1.0)
                wn = gpool.tile([128, BT], BF16, tag="wn")
                nc.vector.tensor_mul(wn, st["wexp"], rz)
                st["wn"] = wn

            def phA2(l, t, st):
                """C matmuls + tanh."""
                pc0 = ps_s.tile([128, BT], F32, tag="ps")
                nc.tensor.matmul(pc0, (cw[:, l, 0, :]), (st["v1_0"]), start=True, stop=True)
                v2_0 = vpool.tile([128, BT], BF16, tag="v2_0")
                nc.scalar.activation(v2_0, pc0, TanhF)
                pc1 = ps_s.tile([128, BT], F32, tag="ps")
                nc.tensor.matmul(pc1, (cw[:, l, 1, :]), (st["v1_1"]), start=True, stop=True)
                v2_1 = vpool.tile([128, BT], BF16, tag="v2_1")
                nc.scalar.activation(v2_1, pc1, TanhF)
                st["v2_0"], st["v2_1"] = v2_0, v2_1

            def phU(l, t, v2s, cps, nxt):
                """U-stage accumulate + state update for chunk-pairs cps."""
                a = aas[t]
                x0 = x0s[t]
                for cp in cps:
                    pm = ps_mc.tile([128, 2, BT], F32, tag="mc")
                    for j in range(2):
                        c = 2 * cp + j
                        nc.tensor.matmul(pm[:, j, :], (uw[:, l, 0, c, :]), (v2s[0]),
                                         start=True, stop=False)
                        nc.tensor.matmul(pm[:, j, :], (uw[:, l, 1, c, :]), (v2s[1]),
                                         start=False, stop=True)
                    asl = a[:, 2 * cp:2 * cp + 2, :]
                    if l == 0:
                        nc.vector.tensor_scalar_add(asl, pm, 1.0)
                    else:
                        nc.vector.tensor_add(asl, asl, pm)
                    eng = nc.vector if cp < 2 else nc.gpsimd
                    eng.tensor_mul(nxt[:, 2 * cp:2 * cp + 2, :], asl,
                                   x0[:, 2 * cp:2 * cp + 2, :])
                    if l == L - 1:
                        nc.sync.dma_start(
                            out=yt[:, t * DC * BT + 2 * cp * BT:
                                   t * DC * BT + (2 * cp + 2) * BT].rearrange(
                                       "q (c b) -> q c b", c=2),
                            in_=nxt[:, 2 * cp:2 * cp + 2, :],
                        )

            items = [
                (l, g * GRP + ti)
                for g in range(NT // GRP)
                for l in range(L)
                for ti in range(GRP)
            ]
            stA = {}
            stB = {}
            nxts = {}
            n = len(items)
            for i in range(n + 2):
                if i < n:
                    stA[items[i]] = phA1(*items[i])
                    if i == n - 1:
                        # final item: run its softmax chain immediately — no
                        # later A-phase exists to cover the drain latency
                        phZ(*items[i], stA[items[i]])
                        softmax_tail(stA[items[i]])
                if i == 1:
                    # layer-1/2 weights arrive behind the startup-critical DMAs
                    nc.sync.dma_start(out=vw_f[:, LW:], in_=wv[:, LW:])
                    nc.sync.dma_start(out=uw_f[:, LW:], in_=wu[:, LW:])
                if i >= 2:
                    lu, tu = items[i - 2]
                    nxts[tu] = mpool.tile([128, DC, BT], BF16, tag="xm", name="xm")
                    phU(lu, tu, stB[items[i - 2]], [0], nxts[tu])
                # narrow-shape matmuls (Z of item i, gate broadcast of item
                # i-1) grouped to minimize PE tile-config switches
                if i < n - 1:
                    phZ(*items[i], stA[items[i]])
                if 1 <= i < n + 1:
                    key = items[i - 1]
                    stB[key] = phB(*key, stA[key])
                if i >= 2:
                    phU(lu, tu, stB[items[i - 2]], [1], nxts[tu])
                if i < n:
                    phA2(*items[i], stA[items[i]])
                    # softmax tail after the C tanhs: wn is only needed by the
                    # NEXT item's gate broadcast, while the v2 tanhs gate this
                    # item's psum-slot rotation
                    if i < n - 1:
                        softmax_tail(stA[items[i]])
                if i >= 2:
                    key = items[i - 2]
                    phU(lu, tu, stB.pop(key), [2, 3], nxts[tu])
                    stA.pop(key, None)
                    if lu < L - 1:
                        curs[tu] = nxts.pop(tu)
    return nc


_split_ctr = [0]


def split_multi_waits(nc):
    """This walrus build accepts only one sync-wait per instruction; hoist
    extra waits onto same-engine NoOps placed just before the instruction."""
    for f in nc.m.functions:
        for bb in f.blocks:
            insts = list(bb.instructions)
            new = []
            changed = False
            for inst in insts:
                si = inst.sync_info
                if si is not None and si.on_wait is not None and len(si.on_wait) > 1:
                    waits = list(si.on_wait)
                    for w in waits[:-1]:
                        _split_ctr[0] += 1
                        nop = mybir.InstNoOp(
                            name=f"I-waitsplit-{_split_ctr[0]}", ins=[], outs=[]
                        )
                        nop.engine = inst.engine
                        nop.sync_info = mybir.SyncInfo(on_wait=[w], on_update=[])
                        new.append(nop)
                    si.on_wait = waits[-1:]
                    changed = True
                new.append(inst)
            if changed:
                bb.instructions = new


def _host_weights(U, V, C, gating_w, bias):
    """Pack params into partition-major SBUF layouts (see build_nc tiles)."""
    # vw[q, l, p, c, m] = V[l, 2p + m//64, c*128+q, m%64]
    Vt = V.reshape(L, 2, 2, D, R)                       # [l, p, eloc, d, r]
    vw = np.zeros((128, L, 2, DC, 128), np.float32)
    vv = Vt.transpose(3, 0, 1, 2, 4).reshape(D, L, 2, 128)   # [d, l, p, (eloc r)]
    vw[:] = vv.reshape(DC, 128, L, 2, 128).transpose(1, 2, 3, 0, 4)
    # gw[q, l, c, e] = gating_w[e, c*128+q]
    gw = np.ascontiguousarray(
        gating_w.T.reshape(DC, 128, E).transpose(1, 0, 2))   # [q, c, e]
    # cw[q, l, p, m]: block-diag of C[l,2p].T, C[l,2p+1].T
    cw = np.zeros((128, L, 2, 128), np.float32)
    for l in range(L):
        for p in range(2):
            for el in range(2):
                cw[el * 64:(el + 1) * 64, l, p, el * 64:(el + 1) * 64] = C[l, 2 * p + el].T
    # uw[q, l, k, c, m] = U[l, 2k + q//64, c*128+m, q%64]
    Ut = U.reshape(L, 2, 2, D, R)                       # [l, k, eloc, d, r]
    uu = Ut.transpose(2, 4, 0, 1, 3).reshape(128, L, 2, D)   # [(eloc r), l, k, d]
    uw = np.ascontiguousarray(uu.reshape(128, L, 2, DC, 128))
    # sel[e, p, m] = 1 if 2p + m//64 == e (rows 4..127 stay zero)
    sel = np.zeros((128, 2, 128), np.float32)
    for p in range(2):
        for el in range(2):
            sel[2 * p + el, p, el * 64:(el + 1) * 64] = 1.0
    ones44 = np.zeros((128, 128), np.float32)
    ones44[:E, :] = 1.0
    # one pad row carries eps so pz pad rows are positive (finite ln/exp)
    ones44[E, :] = 1e-30
    return {
        "wv": np.ascontiguousarray(vw.reshape(128, -1)).astype(NP_BF16),
        "wg": np.ascontiguousarray(gw.reshape(128, -1)).astype(NP_BF16),
        "wc": np.ascontiguousarray(cw.reshape(128, -1)).astype(NP_BF16),
        "wu": np.ascontiguousarray(uw.reshape(128, -1)).astype(NP_BF16),
        "wsel": np.ascontiguousarray(sel.reshape(128, -1)).astype(NP_BF16),
        "ones44": ones44.astype(NP_BF16),
    }


_cache = {}


def kernel(inputs, U, V, C, gating_w, bias):
    inputs = np.asarray(inputs, np.float32)
    U, V, C = np.asarray(U, np.float32), np.asarray(V, np.float32), np.asarray(C, np.float32)
    gating_w, bias = np.asarray(gating_w, np.float32), np.asarray(bias, np.float32)
    assert not np.any(bias), "kernel assumes zero bias"

    if "nc" not in _cache:
        nc = build_nc()
        split_multi_waits(nc)
        _cache["nc"] = nc
    nc = _cache["nc"]

    wmap = _host_weights(U, V, C, gating_w, bias)
    in_maps = []
    for k in range(N_CORES):
        # xt[q, t, c, b] = x[c*128+q, t*BT+b] for this core's rows
        xk = inputs[k * BC:(k + 1) * BC].T.astype(NP_BF16)     # [D, BC]
        xk = xk.reshape(DC, 128, NT, BT).transpose(1, 2, 0, 3)  # [q, t, c, b]
        in_maps.append({"xt": np.ascontiguousarray(xk.reshape(128, -1)), **wmap})

    res = run_bass_kernel_spmd(
        nc, in_maps, core_ids=list(range(N_CORES)),
        trace=bool(_cache.get("trace")),
    )
    _cache["last_result"] = res
    out = np.empty((B, D), np.float32)
    for k in range(N_CORES):
        yk = res.results[k]["yt"].astype(np.float32)           # [128, NT*DC*BT]
        yk = yk.reshape(128, NT, DC, BT).transpose(2, 0, 1, 3)  # [c, q, t, b]
        out[k * BC:(k + 1) * BC] = yk.reshape(D, BC).T
    return out
